# revision 7
# baseline (speedup 1.0000x reference)
"""MoE expert-parallel kernel for Trainium2 (8 NeuronCores, 1 expert/core).

Reference computation per expert e:
    h   = relu(x_e @ W1_e)               [N, DFF]
    agg[d] += h[src[k]] for dst[k]==d    (segment-sum over NE edges)
    out = agg @ W2_e                     [N, D]

Key transformations:
  1. segment_sum is linear:  (S @ h) @ W2 == S @ (h @ W2),
     where S[d, s] = #edges s->d.  Applying W2 *before* the aggregation
     halves the cost of the aggregation matmul (D < DFF).
  2. S is built on the host from edge_index (dense count matrix) so the
     gather/scatter becomes a dense matmul on the tensor engine.
  3. Everything runs in bf16 (full-rate on the PE, exact for the small
     integer counts in S, ~3e-3 relative error overall) with fp32 PSUM
     accumulation.  bf16 halves DMA traffic and SBUF footprint vs fp32,
     which lets the whole intermediate state stay SBUF-resident:

Device pipeline per core (expert), fully fused:
    for each 512-token slice (8 slices):
      phase A: h[f, n]  = relu( W1[d, f].T @ x[d, n] )   (K = D, h in SBUF)
      phase B: m[n, d]  = h[f, n].T @ W2[f, d]           (K = DFF, m in SBUF)
    phase C:   out[n', d] = ST[s, n'].T @ m[s, d]        (K = N)
h never leaves SBUF (no DRAM round-trip), m is fully SBUF-resident in
bf16 (8MB).  W1/W2 are staged just-in-time during the first slice's
phase A; ST tiles stream from DRAM during phase C, double-buffered.
"""

import os

import numpy as np
import ml_dtypes

import concourse.bass as bass
import concourse.mybir as mybir
import concourse.tile as tile
from concourse import bacc
from concourse.bass_utils import run_bass_kernel_spmd

E, N, D, DFF = 8, 4096, 1024, 2048
P = 128
NT = N // P     # 32  n tiles of 128 tokens
DC = D // P     # 8   d chunks (K for phase A)
FT = DFF // P   # 16  f tiles (K chunks for phase B)
NS = N // 512   # 8   n slices of 512
SPT = 4         # 128-token tiles per 512 slice
DS = D // 512   # 2   d slices of 512

F32 = mybir.dt.float32
BF16 = mybir.dt.bfloat16

_cache = {}


def _emit(nc, tc):
    XH = nc.dram_tensor("XH", [NS, P, DC, 512], BF16, kind="ExternalInput")
    # W1H[ft, p, dc, f'] = W1[dc*128 + p, ft*128 + f']
    W1H = nc.dram_tensor("W1H", [FT, P, DC, P], BF16, kind="ExternalInput")
    # W2H[p, fc, d] = W2[fc*128 + p, d]
    W2H = nc.dram_tensor("W2H", [P, FT, D], BF16, kind="ExternalInput")
    # STH[nt, p, sc, n'] = S_T[sc*128 + p, nt*128 + n']
    STH = nc.dram_tensor("STH", [NT, P, NT, P], BF16, kind="ExternalInput")
    out = nc.dram_tensor("out", [N, D], F32, kind="ExternalOutput")

    with tc.tile_pool(name="w1p", bufs=1) as w1p, \
         tc.tile_pool(name="w2p", bufs=1) as w2p, \
         tc.tile_pool(name="mp", bufs=1) as mp:
        w1sb = w1p.tile([P, FT, DC, P], BF16, name="w1sb")
        w2sb = w2p.tile([P, FT, D], BF16, name="w2sb")
        msb = [mp.tile([P, D], BF16, tag=f"m{nt}", name=f"msb{nt}") for nt in range(NT)]

        # ---------- phases A+B, interleaved per 512-token slice ----------
        with tc.tile_pool(name="xp", bufs=2) as xp, \
             tc.tile_pool(name="hp", bufs=2) as hp, \
             tc.tile_pool(name="psA", bufs=4, space="PSUM") as psA, \
             tc.tile_pool(name="psB", bufs=4, space="PSUM") as psB:
            xsbs = [xp.tile([P, DC, 512], BF16, tag="xsb", name=f"xsb{ns}")
                    for ns in range(NS)]
            hsbs = [hp.tile([P, FT, 512], BF16, tag="hsb", name=f"hsb{ns}")
                    for ns in range(NS)]
            # HAM warm-up: ~16 dummy matmuls on a memset tile keep the PE
            # busy through the input-staging preamble, so the SHORT window
            # fires and the first real matmuls run at 2.4 GHz instead of 1.2
            warm = xp.tile([P, 640], BF16, tag="warm", name="warm")
            nc.vector.memset(warm[:], 1.0)
            wpt = psA.tile([P, 512], F32, tag="ptA", name="ptA")
            for i in range(12):
                nc.tensor.matmul(
                    out=wpt[:],
                    lhsT=warm[:, 0:128],
                    rhs=warm[:, 128:640],
                    start=(i == 0),
                    stop=(i == 11),
                )
            # critical path for the first A groups: x slice 0 + W1 f-tiles
            # 0-1 only (~1.5MB).  Everything else (W2, x1, later W1 tiles)
            # stages after ft>=1 so it doesn't contend.
            nc.sync.dma_start(out=xsbs[0][:, :, 0:256], in_=XH[0][:, :, 0:256])
            nc.sync.dma_start(out=xsbs[0][:, :, 256:512], in_=XH[0][:, :, 256:512])
            nc.sync.dma_start(out=w1sb[:, 0], in_=W1H[0])
            nc.sync.dma_start(out=w1sb[:, 1], in_=W1H[1])
            for ns in range(NS):
                # phase A: h = relu(W1.T @ x) for this slice
                for ft in range(FT):
                    pt = psA.tile([P, 512], F32, name="ptA")
                    for dc in range(DC):
                        nc.tensor.matmul(
                            out=pt[:],
                            lhsT=w1sb[:, ft, dc, :],
                            rhs=xsbs[ns][:, dc, :],
                            start=(dc == 0),
                            stop=(dc == DC - 1),
                        )
                    # relu on the DVE (immediate scalar, no const-AP preamble)
                    nc.vector.tensor_scalar_max(
                        out=hsbs[ns][:, ft, :], in0=pt[:], scalar1=0.0
                    )
                    if ns == 0 and 1 <= ft < FT - 1:
                        # JIT-stage the next W1 chunk behind this group
                        nc.sync.dma_start(out=w1sb[:, ft + 1], in_=W1H[ft + 1])
                    if ns == 0 and 2 <= ft < 2 + FT // 2:
                        # stage W2 during slice 0's phase A (two chunks per
                        # group from ft=2; B0 needs the full W2)
                        fc = 2 * (ft - 2)
                        nc.sync.dma_start(out=w2sb[:, fc], in_=W2H[:, fc])
                        nc.sync.dma_start(out=w2sb[:, fc + 1], in_=W2H[:, fc + 1])
                    if ns + 1 < NS and ft == (3 if ns == 0 else 0):
                        nc.sync.dma_start(out=xsbs[ns + 1][:], in_=XH[ns + 1])
                # phase B: m = h.T @ W2 for this slice
                for ntl in range(SPT):
                    nt = ns * SPT + ntl
                    for ds in range(DS):
                        pt = psB.tile([P, 512], F32, name="ptB")
                        for fc in range(FT):
                            nc.tensor.matmul(
                                out=pt[:],
                                lhsT=hsbs[ns][:, fc, ntl * P:(ntl + 1) * P],
                                rhs=w2sb[:, fc, ds * 512:(ds + 1) * 512],
                                start=(fc == 0),
                                stop=(fc == FT - 1),
                            )
                        nc.vector.tensor_copy(
                            out=msb[nt][:, ds * 512:(ds + 1) * 512], in_=pt[:]
                        )

        # ---------- phase C: out = ST.T @ m ----------
        with tc.tile_pool(name="stp", bufs=2) as stp, \
             tc.tile_pool(name="op", bufs=4) as op, \
             tc.tile_pool(name="psC", bufs=4, space="PSUM") as psC:
            stsbs = [stp.tile([P, NT, P], BF16, tag="stsb", name=f"stsb{nt}")
                     for nt in range(NT)]
            nc.sync.dma_start(out=stsbs[0][:], in_=STH[0])
            for nt in range(NT):
                if nt + 1 < NT:
                    nc.sync.dma_start(out=stsbs[nt + 1][:], in_=STH[nt + 1])
                for ds in range(DS):
                    pt = psC.tile([P, 512], F32, name="ptC")
                    for sc in range(NT):
                        nc.tensor.matmul(
                            out=pt[:],
                            lhsT=stsbs[nt][:, sc, :],
                            rhs=msb[sc][:, ds * 512:(ds + 1) * 512],
                            start=(sc == 0),
                            stop=(sc == NT - 1),
                        )
                    osb = op.tile([P, 512], F32, name="osb")
                    nc.vector.tensor_copy(out=osb[:], in_=pt[:])
                    nc.sync.dma_start(
                        out=out[nt * P:(nt + 1) * P, ds * 512:(ds + 1) * 512],
                        in_=osb[:],
                    )


def _build():
    nc = bacc.Bacc()
    with tile.TileContext(nc) as tc:
        _emit(nc, tc)
    nc.compile()
    return nc


def kernel(x, W1, W2, edge_index):
    bf = ml_dtypes.bfloat16
    x = np.asarray(x, dtype=np.float32)
    W1 = np.asarray(W1, dtype=np.float32)
    W2 = np.asarray(W2, dtype=np.float32)
    edge_index = np.asarray(edge_index)

    # S_T[s, d] = #edges with src==s and dst==d  (so out = S_T.T @ m)
    src = edge_index[0].astype(np.int64)
    dst = edge_index[1].astype(np.int64)
    counts = np.bincount(src * N + dst, minlength=N * N)
    S_T = counts.reshape(N, N)
    # host tiling for contiguous phase-C DMA: [nt, p, sc, n']; counts are
    # small ints -> exact in bf16
    STH = np.ascontiguousarray(
        S_T.reshape(NT, P, NT, P).transpose(2, 1, 0, 3).astype(bf)
    )

    if "nc" not in _cache:
        _cache["nc"] = _build()
    nc = _cache["nc"]

    in_maps = []
    for e in range(E):
        # XH[ns, p, dc, j] = x[e, ns*512 + j, dc*128 + p]
        XH = np.ascontiguousarray(
            x[e].reshape(NS, 512, DC, P).transpose(0, 3, 2, 1).astype(bf)
        )
        # W1H[ft, p, dc, f'] = W1[e, dc*128 + p, ft*128 + f']
        W1H = np.ascontiguousarray(
            W1[e].reshape(DC, P, FT, P).transpose(2, 1, 0, 3).astype(bf)
        )
        # W2H[p, fc, d] = W2[e, fc*128 + p, d]
        W2H = np.ascontiguousarray(
            W2[e].reshape(FT, P, D).transpose(1, 0, 2).astype(bf)
        )
        in_maps.append({"XH": XH, "W1H": W1H, "W2H": W2H, "STH": STH})

    trace = bool(int(os.environ.get("PROBLEM_TRACE", "0")))
    res = run_bass_kernel_spmd(nc, in_maps, core_ids=list(range(E)), trace=trace)
    _cache["last_results"] = res
    return np.stack([res.results[e]["out"] for e in range(E)]).astype(np.float32)


# revision 10
# speedup vs baseline: 1.0006x; 1.0006x over previous
"""MoE expert-parallel kernel for Trainium2 (8 NeuronCores, 1 expert/core).

Reference computation per expert e:
    h   = relu(x_e @ W1_e)               [N, DFF]
    agg[d] += h[src[k]] for dst[k]==d    (segment-sum over NE edges)
    out = agg @ W2_e                     [N, D]

Key transformations:
  1. segment_sum is linear:  (S @ h) @ W2 == S @ (h @ W2),
     where S[d, s] = #edges s->d.  Applying W2 *before* the aggregation
     halves the cost of the aggregation matmul (D < DFF).
  2. S is built on the host from edge_index (dense count matrix) so the
     gather/scatter becomes a dense matmul on the tensor engine.
  3. Everything runs in bf16 (full-rate on the PE, exact for the small
     integer counts in S, ~3e-3 relative error overall) with fp32 PSUM
     accumulation.  bf16 halves DMA traffic and SBUF footprint vs fp32,
     which lets the whole intermediate state stay SBUF-resident:

Device pipeline per core (expert), fully fused:
    for each 512-token slice (8 slices):
      phase A: h[f, n]  = relu( W1[d, f].T @ x[d, n] )   (K = D, h in SBUF)
      phase B: m[n, d]  = h[f, n].T @ W2[f, d]           (K = DFF, m in SBUF)
    phase C:   out[n', d] = ST[s, n'].T @ m[s, d]        (K = N)
h never leaves SBUF (no DRAM round-trip), m is fully SBUF-resident in
bf16 (8MB).  W1/W2 are staged just-in-time during the first slice's
phase A; ST tiles stream from DRAM during phase C, double-buffered.

The PE runs the 4096 512-row matmuls back-to-back with zero gaps at the
warm 216ns rate (measured); a dozen dummy matmuls on a memset tile bridge
the input-staging preamble so the HAM clock-gate releases (1.2 -> 2.4
GHz) before the first real matmul.  Relu runs on the DVE via
tensor_scalar_max with an immediate 0.0 (no const-AP preamble).
Measured ~0.91 ms/kernel vs the ~0.88 ms pure-PE roofline.
"""

import os

import numpy as np
import ml_dtypes

import concourse.mybir as mybir
import concourse.tile as tile
from concourse import bacc
from concourse.bass_utils import run_bass_kernel_spmd

E, N, D, DFF = 8, 4096, 1024, 2048
P = 128
NT = N // P     # 32  n tiles of 128 tokens
DC = D // P     # 8   d chunks (K for phase A)
FT = DFF // P   # 16  f tiles (K chunks for phase B)
NS = N // 512   # 8   n slices of 512
SPT = 4         # 128-token tiles per 512 slice
DS = D // 512   # 2   d slices of 512

F32 = mybir.dt.float32
BF16 = mybir.dt.bfloat16

_cache = {}


def _emit(nc, tc):
    XH = nc.dram_tensor("XH", [NS, P, DC, 512], BF16, kind="ExternalInput")
    # W1H[ft, p, dc, f'] = W1[dc*128 + p, ft*128 + f']
    W1H = nc.dram_tensor("W1H", [FT, P, DC, P], BF16, kind="ExternalInput")
    # W2H[p, fc, d] = W2[fc*128 + p, d]
    W2H = nc.dram_tensor("W2H", [P, FT, D], BF16, kind="ExternalInput")
    # STH[nt, p, sc, n'] = S_T[sc*128 + p, nt*128 + n']
    STH = nc.dram_tensor("STH", [NT, P, NT, P], BF16, kind="ExternalInput")
    out = nc.dram_tensor("out", [N, D], F32, kind="ExternalOutput")

    with tc.tile_pool(name="w1p", bufs=1) as w1p, \
         tc.tile_pool(name="w2p", bufs=1) as w2p, \
         tc.tile_pool(name="mp", bufs=1) as mp:
        w1sb = w1p.tile([P, FT, DC, P], BF16, name="w1sb")
        w2sb = w2p.tile([P, FT, D], BF16, name="w2sb")
        msb = [mp.tile([P, D], BF16, tag=f"m{nt}", name=f"msb{nt}") for nt in range(NT)]

        # ---------- phases A+B, interleaved per 512-token slice ----------
        with tc.tile_pool(name="xp", bufs=2) as xp, \
             tc.tile_pool(name="hp", bufs=2) as hp, \
             tc.tile_pool(name="psA", bufs=4, space="PSUM") as psA, \
             tc.tile_pool(name="psB", bufs=4, space="PSUM") as psB:
            xsbs = [xp.tile([P, DC, 512], BF16, tag="xsb", name=f"xsb{ns}")
                    for ns in range(NS)]
            hsbs = [hp.tile([P, FT, 512], BF16, tag="hsb", name=f"hsb{ns}")
                    for ns in range(NS)]
            # HAM warm-up: ~16 dummy matmuls on a memset tile keep the PE
            # busy through the input-staging preamble, so the SHORT window
            # fires and the first real matmuls run at 2.4 GHz instead of 1.2
            warm = xp.tile([P, 640], BF16, tag="warm", name="warm")
            nc.vector.memset(warm[:], 1.0)
            wpt = psA.tile([P, 512], F32, tag="ptA", name="ptA")
            for i in range(12):
                nc.tensor.matmul(
                    out=wpt[:],
                    lhsT=warm[:, 0:128],
                    rhs=warm[:, 128:640],
                    start=(i == 0),
                    stop=(i == 11),
                )
            # critical path for the first A groups: x slice 0 + W1 f-tiles
            # 0-1 only (~1.5MB).  Everything else (W2, x1, later W1 tiles)
            # stages after ft>=1 so it doesn't contend.
            nc.sync.dma_start(out=xsbs[0][:, :, 0:256], in_=XH[0][:, :, 0:256])
            nc.sync.dma_start(out=xsbs[0][:, :, 256:512], in_=XH[0][:, :, 256:512])
            nc.sync.dma_start(out=w1sb[:, 0], in_=W1H[0])
            nc.sync.dma_start(out=w1sb[:, 1], in_=W1H[1])
            for ns in range(NS):
                # phase A: h = relu(W1.T @ x) for this slice
                for ft in range(FT):
                    pt = psA.tile([P, 512], F32, name="ptA")
                    for dc in range(DC):
                        nc.tensor.matmul(
                            out=pt[:],
                            lhsT=w1sb[:, ft, dc, :],
                            rhs=xsbs[ns][:, dc, :],
                            start=(dc == 0),
                            stop=(dc == DC - 1),
                        )
                    # relu on the DVE (immediate scalar, no const-AP preamble)
                    nc.vector.tensor_scalar_max(
                        out=hsbs[ns][:, ft, :], in0=pt[:], scalar1=0.0
                    )
                    if ns == 0 and 1 <= ft < FT - 1:
                        # JIT-stage the next W1 chunk behind this group
                        nc.sync.dma_start(out=w1sb[:, ft + 1], in_=W1H[ft + 1])
                    if ns == 0 and 2 <= ft < 2 + FT // 2:
                        # stage W2 during slice 0's phase A (two chunks per
                        # group from ft=2; B0 needs the full W2)
                        fc = 2 * (ft - 2)
                        nc.sync.dma_start(out=w2sb[:, fc], in_=W2H[:, fc])
                        nc.sync.dma_start(out=w2sb[:, fc + 1], in_=W2H[:, fc + 1])
                    if ns + 1 < NS and ft == (3 if ns == 0 else 0):
                        nc.sync.dma_start(out=xsbs[ns + 1][:], in_=XH[ns + 1])
                # phase B: m = h.T @ W2 for this slice
                for ntl in range(SPT):
                    nt = ns * SPT + ntl
                    for ds in range(DS):
                        pt = psB.tile([P, 512], F32, name="ptB")
                        for fc in range(FT):
                            nc.tensor.matmul(
                                out=pt[:],
                                lhsT=hsbs[ns][:, fc, ntl * P:(ntl + 1) * P],
                                rhs=w2sb[:, fc, ds * 512:(ds + 1) * 512],
                                start=(fc == 0),
                                stop=(fc == FT - 1),
                            )
                        nc.vector.tensor_copy(
                            out=msb[nt][:, ds * 512:(ds + 1) * 512], in_=pt[:]
                        )

        # ---------- phase C: out = ST.T @ m ----------
        with tc.tile_pool(name="stp", bufs=2) as stp, \
             tc.tile_pool(name="op", bufs=4) as op, \
             tc.tile_pool(name="psC", bufs=6, space="PSUM") as psC:
            stsbs = [stp.tile([P, NT, P], BF16, tag="stsb", name=f"stsb{nt}")
                     for nt in range(NT)]
            nc.sync.dma_start(out=stsbs[0][:], in_=STH[0])
            for nt in range(NT):
                if nt + 1 < NT:
                    nc.sync.dma_start(out=stsbs[nt + 1][:], in_=STH[nt + 1])
                for ds in range(DS):
                    pt = psC.tile([P, 512], F32, name="ptC")
                    for sc in range(NT):
                        nc.tensor.matmul(
                            out=pt[:],
                            lhsT=stsbs[nt][:, sc, :],
                            rhs=msb[sc][:, ds * 512:(ds + 1) * 512],
                            start=(sc == 0),
                            stop=(sc == NT - 1),
                        )
                    osb = op.tile([P, 512], F32, name="osb")
                    nc.vector.tensor_copy(out=osb[:], in_=pt[:])
                    nc.sync.dma_start(
                        out=out[nt * P:(nt + 1) * P, ds * 512:(ds + 1) * 512],
                        in_=osb[:],
                    )


def _build():
    nc = bacc.Bacc()
    with tile.TileContext(nc) as tc:
        _emit(nc, tc)
    nc.compile()
    return nc


def kernel(x, W1, W2, edge_index):
    bf = ml_dtypes.bfloat16
    x = np.asarray(x, dtype=np.float32)
    W1 = np.asarray(W1, dtype=np.float32)
    W2 = np.asarray(W2, dtype=np.float32)
    edge_index = np.asarray(edge_index)

    # S_T[s, d] = #edges with src==s and dst==d  (so out = S_T.T @ m)
    src = edge_index[0].astype(np.int64)
    dst = edge_index[1].astype(np.int64)
    counts = np.bincount(src * N + dst, minlength=N * N)
    S_T = counts.reshape(N, N)
    # host tiling for contiguous phase-C DMA: [nt, p, sc, n']; counts are
    # small ints -> exact in bf16
    STH = np.ascontiguousarray(
        S_T.reshape(NT, P, NT, P).transpose(2, 1, 0, 3).astype(bf)
    )

    if "nc" not in _cache:
        _cache["nc"] = _build()
    nc = _cache["nc"]

    in_maps = []
    for e in range(E):
        # XH[ns, p, dc, j] = x[e, ns*512 + j, dc*128 + p]
        XH = np.ascontiguousarray(
            x[e].reshape(NS, 512, DC, P).transpose(0, 3, 2, 1).astype(bf)
        )
        # W1H[ft, p, dc, f'] = W1[e, dc*128 + p, ft*128 + f']
        W1H = np.ascontiguousarray(
            W1[e].reshape(DC, P, FT, P).transpose(2, 1, 0, 3).astype(bf)
        )
        # W2H[p, fc, d] = W2[e, fc*128 + p, d]
        W2H = np.ascontiguousarray(
            W2[e].reshape(FT, P, D).transpose(1, 0, 2).astype(bf)
        )
        in_maps.append({"XH": XH, "W1H": W1H, "W2H": W2H, "STH": STH})

    trace = bool(int(os.environ.get("PROBLEM_TRACE", "0")))
    res = run_bass_kernel_spmd(nc, in_maps, core_ids=list(range(E)), trace=trace)
    _cache["last_results"] = res
    return np.stack([res.results[e]["out"] for e in range(E)]).astype(np.float32)


# revision 11
# speedup vs baseline: 1.0032x; 1.0026x over previous
"""MoE expert-parallel kernel for Trainium2 (8 NeuronCores, 1 expert/core).

Reference computation per expert e:
    h   = relu(x_e @ W1_e)               [N, DFF]
    agg[d] += h[src[k]] for dst[k]==d    (segment-sum over NE edges)
    out = agg @ W2_e                     [N, D]

Key transformations:
  1. segment_sum is linear:  (S @ h) @ W2 == S @ (h @ W2),
     where S[d, s] = #edges s->d.  Applying W2 *before* the aggregation
     halves the cost of the aggregation matmul (D < DFF).
  2. S is built on the host from edge_index (dense count matrix) so the
     gather/scatter becomes a dense matmul on the tensor engine.
  3. Everything runs in bf16 (full-rate on the PE, exact for the small
     integer counts in S, ~3e-3 relative error overall) with fp32 PSUM
     accumulation.  bf16 halves DMA traffic and SBUF footprint vs fp32,
     which lets the whole intermediate state stay SBUF-resident:

Device pipeline per core (expert), fully fused:
    for each 512-token slice (8 slices):
      phase A: h[f, n]  = relu( W1[d, f].T @ x[d, n] )   (K = D, h in SBUF)
      phase B: m[n, d]  = h[f, n].T @ W2[f, d]           (K = DFF, m in SBUF)
    phase C:   out[n', d] = ST[s, n'].T @ m[s, d]        (K = N)
h never leaves SBUF (no DRAM round-trip), m is fully SBUF-resident in
bf16 (8MB).  W1/W2 are staged just-in-time during the first slice's
phase A; ST tiles stream from DRAM during phase C, double-buffered.

The PE runs the 4096 512-row matmuls back-to-back with zero gaps at the
warm 216ns rate (measured); a dozen dummy matmuls on a memset tile bridge
the input-staging preamble so the HAM clock-gate releases (1.2 -> 2.4
GHz) before the first real matmul.  Relu runs on the DVE via
tensor_scalar_max with an immediate 0.0 (no const-AP preamble).
Measured ~0.91 ms/kernel vs the ~0.88 ms pure-PE roofline.
"""

import os

import numpy as np
import ml_dtypes

import concourse.mybir as mybir
import concourse.tile as tile
from concourse import bacc
from concourse.bass_utils import run_bass_kernel_spmd

E, N, D, DFF = 8, 4096, 1024, 2048
P = 128
NT = N // P     # 32  n tiles of 128 tokens
DC = D // P     # 8   d chunks (K for phase A)
FT = DFF // P   # 16  f tiles (K chunks for phase B)
NS = N // 512   # 8   n slices of 512
SPT = 4         # 128-token tiles per 512 slice
DS = D // 512   # 2   d slices of 512

F32 = mybir.dt.float32
BF16 = mybir.dt.bfloat16

_cache = {}


def _emit(nc, tc):
    XH = nc.dram_tensor("XH", [NS, P, DC, 512], BF16, kind="ExternalInput")
    # W1H[ft, p, dc, f'] = W1[dc*128 + p, ft*128 + f']
    W1H = nc.dram_tensor("W1H", [FT, P, DC, P], BF16, kind="ExternalInput")
    # W2H[p, fc, d] = W2[fc*128 + p, d]
    W2H = nc.dram_tensor("W2H", [P, FT, D], BF16, kind="ExternalInput")
    # STH[nt, p, sc, n'] = S_T[sc*128 + p, nt*128 + n']
    STH = nc.dram_tensor("STH", [NT, P, NT, P], BF16, kind="ExternalInput")
    out = nc.dram_tensor("out", [N, D], F32, kind="ExternalOutput")

    with tc.tile_pool(name="w1p", bufs=1) as w1p, \
         tc.tile_pool(name="w2p", bufs=1) as w2p, \
         tc.tile_pool(name="mp", bufs=1) as mp:
        w1sb = w1p.tile([P, FT, DC, P], BF16, name="w1sb")
        w2sb = w2p.tile([P, FT, D], BF16, name="w2sb")
        msb = [mp.tile([P, D], BF16, tag=f"m{nt}", name=f"msb{nt}") for nt in range(NT)]

        # ---------- phases A+B, interleaved per 512-token slice ----------
        with tc.tile_pool(name="xp", bufs=2) as xp, \
             tc.tile_pool(name="hp", bufs=2) as hp, \
             tc.tile_pool(name="psA", bufs=4, space="PSUM") as psA, \
             tc.tile_pool(name="psB", bufs=4, space="PSUM") as psB:
            xsbs = [xp.tile([P, DC, 512], BF16, tag="xsb", name=f"xsb{ns}")
                    for ns in range(NS)]
            hsbs = [hp.tile([P, FT, 512], BF16, tag="hsb", name=f"hsb{ns}")
                    for ns in range(NS)]
            # HAM warm-up: ~16 dummy matmuls on a memset tile keep the PE
            # busy through the input-staging preamble, so the SHORT window
            # fires and the first real matmuls run at 2.4 GHz instead of 1.2
            warm = xp.tile([P, 640], BF16, tag="warm", name="warm")
            nc.vector.memset(warm[:], 1.0)
            wpt = psA.tile([P, 512], F32, tag="ptA", name="ptA")
            for i in range(16):
                nc.tensor.matmul(
                    out=wpt[:],
                    lhsT=warm[:, 0:128],
                    rhs=warm[:, 128:640],
                    start=(i == 0),
                    stop=(i == 15),
                )
            # critical path for the first A groups: x slice 0 + W1 f-tiles
            # 0-1 only (~1.5MB).  Everything else (W2, x1, later W1 tiles)
            # stages after ft>=1 so it doesn't contend.
            nc.sync.dma_start(out=xsbs[0][:, :, 0:256], in_=XH[0][:, :, 0:256])
            nc.sync.dma_start(out=xsbs[0][:, :, 256:512], in_=XH[0][:, :, 256:512])
            nc.sync.dma_start(out=w1sb[:, 0], in_=W1H[0])
            nc.sync.dma_start(out=w1sb[:, 1], in_=W1H[1])
            for ns in range(NS):
                # phase A: h = relu(W1.T @ x) for this slice
                for ft in range(FT):
                    pt = psA.tile([P, 512], F32, name="ptA")
                    for dc in range(DC):
                        nc.tensor.matmul(
                            out=pt[:],
                            lhsT=w1sb[:, ft, dc, :],
                            rhs=xsbs[ns][:, dc, :],
                            start=(dc == 0),
                            stop=(dc == DC - 1),
                        )
                    # relu on the DVE (immediate scalar, no const-AP preamble)
                    nc.vector.tensor_scalar_max(
                        out=hsbs[ns][:, ft, :], in0=pt[:], scalar1=0.0
                    )
                    if ns == 0 and 1 <= ft < FT - 1:
                        # JIT-stage the next W1 chunk behind this group
                        nc.sync.dma_start(out=w1sb[:, ft + 1], in_=W1H[ft + 1])
                    if ns == 0 and 2 <= ft < 2 + FT // 2:
                        # stage W2 during slice 0's phase A (two chunks per
                        # group from ft=2; B0 needs the full W2)
                        fc = 2 * (ft - 2)
                        nc.sync.dma_start(out=w2sb[:, fc], in_=W2H[:, fc])
                        nc.sync.dma_start(out=w2sb[:, fc + 1], in_=W2H[:, fc + 1])
                    if ns + 1 < NS and ft == (3 if ns == 0 else 0):
                        nc.sync.dma_start(out=xsbs[ns + 1][:], in_=XH[ns + 1])
                # phase B: m = h.T @ W2 for this slice
                for ntl in range(SPT):
                    nt = ns * SPT + ntl
                    for ds in range(DS):
                        pt = psB.tile([P, 512], F32, name="ptB")
                        for fc in range(FT):
                            nc.tensor.matmul(
                                out=pt[:],
                                lhsT=hsbs[ns][:, fc, ntl * P:(ntl + 1) * P],
                                rhs=w2sb[:, fc, ds * 512:(ds + 1) * 512],
                                start=(fc == 0),
                                stop=(fc == FT - 1),
                            )
                        nc.vector.tensor_copy(
                            out=msb[nt][:, ds * 512:(ds + 1) * 512], in_=pt[:]
                        )

        # ---------- phase C: out = ST.T @ m ----------
        with tc.tile_pool(name="stp", bufs=2) as stp, \
             tc.tile_pool(name="op", bufs=4) as op, \
             tc.tile_pool(name="psC", bufs=6, space="PSUM") as psC:
            stsbs = [stp.tile([P, NT, P], BF16, tag="stsb", name=f"stsb{nt}")
                     for nt in range(NT)]
            nc.sync.dma_start(out=stsbs[0][:], in_=STH[0])
            for nt in range(NT):
                if nt + 1 < NT:
                    nc.sync.dma_start(out=stsbs[nt + 1][:], in_=STH[nt + 1])
                for ds in range(DS):
                    pt = psC.tile([P, 512], F32, name="ptC")
                    for sc in range(NT):
                        nc.tensor.matmul(
                            out=pt[:],
                            lhsT=stsbs[nt][:, sc, :],
                            rhs=msb[sc][:, ds * 512:(ds + 1) * 512],
                            start=(sc == 0),
                            stop=(sc == NT - 1),
                        )
                    osb = op.tile([P, 512], F32, name="osb")
                    nc.vector.tensor_copy(out=osb[:], in_=pt[:])
                    nc.sync.dma_start(
                        out=out[nt * P:(nt + 1) * P, ds * 512:(ds + 1) * 512],
                        in_=osb[:],
                    )


def _build():
    nc = bacc.Bacc()
    with tile.TileContext(nc) as tc:
        _emit(nc, tc)
    nc.compile()
    return nc


def kernel(x, W1, W2, edge_index):
    bf = ml_dtypes.bfloat16
    x = np.asarray(x, dtype=np.float32)
    W1 = np.asarray(W1, dtype=np.float32)
    W2 = np.asarray(W2, dtype=np.float32)
    edge_index = np.asarray(edge_index)

    # S_T[s, d] = #edges with src==s and dst==d  (so out = S_T.T @ m)
    src = edge_index[0].astype(np.int64)
    dst = edge_index[1].astype(np.int64)
    counts = np.bincount(src * N + dst, minlength=N * N)
    S_T = counts.reshape(N, N)
    # host tiling for contiguous phase-C DMA: [nt, p, sc, n']; counts are
    # small ints -> exact in bf16
    STH = np.ascontiguousarray(
        S_T.reshape(NT, P, NT, P).transpose(2, 1, 0, 3).astype(bf)
    )

    if "nc" not in _cache:
        _cache["nc"] = _build()
    nc = _cache["nc"]

    in_maps = []
    for e in range(E):
        # XH[ns, p, dc, j] = x[e, ns*512 + j, dc*128 + p]
        XH = np.ascontiguousarray(
            x[e].reshape(NS, 512, DC, P).transpose(0, 3, 2, 1).astype(bf)
        )
        # W1H[ft, p, dc, f'] = W1[e, dc*128 + p, ft*128 + f']
        W1H = np.ascontiguousarray(
            W1[e].reshape(DC, P, FT, P).transpose(2, 1, 0, 3).astype(bf)
        )
        # W2H[p, fc, d] = W2[e, fc*128 + p, d]
        W2H = np.ascontiguousarray(
            W2[e].reshape(FT, P, D).transpose(1, 0, 2).astype(bf)
        )
        in_maps.append({"XH": XH, "W1H": W1H, "W2H": W2H, "STH": STH})

    trace = bool(int(os.environ.get("PROBLEM_TRACE", "0")))
    res = run_bass_kernel_spmd(nc, in_maps, core_ids=list(range(E)), trace=trace)
    _cache["last_results"] = res
    return np.stack([res.results[e]["out"] for e in range(E)]).astype(np.float32)
